# revision 5
# baseline (speedup 1.0000x reference)
# Trainium2 Bass kernel for nn_FCM_series_1 (gnn_message_passing).
#
# Math (derived from the reference):
#   aggregate(X, WW)[l,b,j] = tanh(-sum_i X[l,b,i] * WW[i,j])
#   T_A  = aggregate(A, WW)                     (12 lags x B rows)
#   U[t] = aggregate(train_init[:,:,t,1], WW)   (13 unique rows per batch;
#          A_N_OLD[la] = U[la], A_0_NEW[la] = U[la+1])
#   out[b,la,j] = P[la,j]*T_A[la,b,j] + Q[la,j]*U[la+1,b,j] + R[la,j]*U[la,b,j]
# with host-computable coefficients
#   P[la,j] = 2 * lambd[la, j%200] / belta[la] * 3**fract[la]
#   Q[la,j] = 3 * lambd[la, j%200] * l[la, j%200] / belta[la]
#   R[la,j] = Q[la,j] * Gamma(a+1)/(6*Gamma(a-2))
#   belta[la] = sum_{k=0..3} Gamma(a+1)/(Gamma(k+1)*Gamma(a-k+1))
#
# Sharding over 8 cores: batch split x2 (16 each), output node dim j split x4
# (300 each). Per core one fp32 matmul [1200,400]^T-layout x [1200,300] done as
# lhsT=W-chunk tiles, rhs=X^T tiles, PSUM-accumulated over 10 k-tiles of 120.
# tanh on ScalarE (W negated on host so psum already holds -X@W), coefficient
# combine on VectorE, [300,192] result per core re-assembled on the host.

import math

import numpy as np

LAG = 13
B = 32
N = 1200
H = 1.0 / 3.0

PB = 2          # batch shards
PJ = 4          # j shards
BL = B // PB    # 16 batches per core
JL = N // PJ    # 300 output nodes per core
NL = LAG - 1    # 12
CA = NL * BL    # 192 cols: T_A block, col = la*BL + b
CU = LAG * BL   # 208 cols: U block,  col = CA + t*BL + b
C = CA + CU     # 400 matmul moving cols
KT = 120        # contraction tile
NK = N // KT    # 10
JS = 100        # j subtile (psum partition dim)
NJ = JL // JS   # 3

_cached = None


def _gamma(x):
    return math.gamma(x)


def _build_nc():
    import concourse.bacc as bacc
    import concourse.mybir as mybir
    from concourse.tile import TileContext

    f32 = mybir.dt.float32
    nc = bacc.Bacc(None, target_bir_lowering=False)

    xt = nc.dram_tensor("xt", [N, C], f32, kind="ExternalInput")
    wc = nc.dram_tensor("wc", [N, JL], f32, kind="ExternalInput")
    pc = nc.dram_tensor("pc", [JL, CA], f32, kind="ExternalInput")
    qc = nc.dram_tensor("qc", [JL, CA], f32, kind="ExternalInput")
    rc = nc.dram_tensor("rc", [JL, CA], f32, kind="ExternalInput")
    out = nc.dram_tensor("out", [JL, CA], f32, kind="ExternalOutput")

    with TileContext(nc) as tc:
        with (
            tc.tile_pool(name="w", bufs=NK) as wpool,
            tc.tile_pool(name="x", bufs=NK) as xpool,
            tc.tile_pool(name="coef", bufs=1) as cpool,
            tc.tile_pool(name="tact", bufs=2) as tpool,
            tc.tile_pool(name="tmp", bufs=2) as mpool,
            tc.tile_pool(name="res", bufs=2) as rpool,
            tc.tile_pool(name="ps", bufs=1, space="PSUM") as pspool,
        ):
            wt, xtt = [], []
            for k in range(NK):
                w_tile = wpool.tile([KT, JL], f32)
                nc.sync.dma_start(out=w_tile[:], in_=wc[k * KT:(k + 1) * KT, :])
                wt.append(w_tile)
                x_tile = xpool.tile([KT, C], f32)
                nc.sync.dma_start(out=x_tile[:], in_=xt[k * KT:(k + 1) * KT, :])
                xtt.append(x_tile)

            coefs = []
            for jt in range(NJ):
                row = []
                for name, dram in (("p", pc), ("q", qc), ("r", rc)):
                    ctile = cpool.tile([JS, CA], f32, tag=f"{name}{jt}")
                    nc.sync.dma_start(
                        out=ctile[:], in_=dram[jt * JS:(jt + 1) * JS, :]
                    )
                    row.append(ctile)
                coefs.append(row)

            ps = [pspool.tile([JS, C], f32, tag=f"ps{jt}", name=f"ps{jt}")
                  for jt in range(NJ)]
            for k in range(NK):
                for jt in range(NJ):
                    nc.tensor.matmul(
                        ps[jt][:],
                        wt[k][:, jt * JS:(jt + 1) * JS],
                        xtt[k][:],
                        start=(k == 0),
                        stop=(k == NK - 1),
                    )

            for jt in range(NJ):
                p_t, q_t, r_t = coefs[jt]
                t = tpool.tile([JS, C], f32, tag="t")
                # W was negated on the host, so psum = -(X @ W) already.
                nc.scalar.activation(
                    out=t[:], in_=ps[jt][:],
                    func=mybir.ActivationFunctionType.Tanh,
                )
                res = rpool.tile([JS, CA], f32, tag="res")
                tmp = mpool.tile([JS, CA], f32, tag="tmp")
                nc.vector.tensor_mul(res[:], p_t[:], t[:, 0:CA])
                nc.vector.tensor_mul(tmp[:], q_t[:], t[:, CA + BL:CA + CU])
                nc.vector.tensor_add(res[:], res[:], tmp[:])
                nc.vector.tensor_mul(tmp[:], r_t[:], t[:, CA:CA + CA])
                nc.vector.tensor_add(res[:], res[:], tmp[:])
                nc.sync.dma_start(out=out[jt * JS:(jt + 1) * JS, :], in_=res[:])

    return nc


def _get_nc():
    global _cached
    if _cached is None:
        _cached = _build_nc()
        _cached.finalize()   # Bacc: runs reg alloc + codegen passes
    return _cached


def _host_coefs(alpha, fract, lambd, l):
    # All [12,...] fp32; compute in float64, cast at the end.
    a = alpha[:, 0].astype(np.float64)          # [12]
    f = fract[:, 0].astype(np.float64)          # [12]
    lam = lambd[:, 0, :, 0].astype(np.float64)  # [12, 200]
    ll = l[:, 0, :, 0].astype(np.float64)       # [12, 200]

    belta = np.zeros(NL)
    for la in range(NL):
        g_a1 = _gamma(a[la] + 1.0)
        belta[la] = sum(
            g_a1 / (_gamma(kk + 1.0) * _gamma(a[la] - kk + 1.0)) for kk in range(4)
        )
    cN = np.array([_gamma(a[la] + 1.0) / (6.0 * _gamma(a[la] - 2.0))
                   for la in range(NL)])

    # tile lambda/l from 200 -> 1200 (index n % 200)
    lam_t = np.tile(lam, (1, 6))                # [12, 1200]
    ll_t = np.tile(ll, (1, 6))                  # [12, 1200]

    inv_hf = (1.0 / H) ** f                     # 3**fract
    P = 2.0 * lam_t / belta[:, None] * inv_hf[:, None]
    Q = lam_t * ll_t / belta[:, None] / H
    R = Q * cN[:, None]
    return P.astype(np.float32), Q.astype(np.float32), R.astype(np.float32)


def _coef_chunk(M, g):
    # M [12, 1200] -> per-core [300, 192] with col = la*BL + b (repeat over b)
    chunk = M[:, g * JL:(g + 1) * JL].T         # [300, 12]
    rep = np.broadcast_to(chunk[:, :, None], (JL, NL, BL))
    return np.ascontiguousarray(rep.reshape(JL, CA), dtype=np.float32)


def kernel(A, WW, train_init, alpha, fract, lambd, l, A_y_list):
    from concourse.bass_utils import run_bass_kernel_spmd

    A = np.asarray(A, dtype=np.float32)
    WW = np.asarray(WW, dtype=np.float32)
    train_init = np.asarray(train_init, dtype=np.float32)

    P, Q, R = _host_coefs(
        np.asarray(alpha, np.float32), np.asarray(fract, np.float32),
        np.asarray(lambd, np.float32), np.asarray(l, np.float32))

    Wneg = -WW[:, :, 0]                         # [1200, 1200]

    in_maps = []
    xts, wcs, pcs, qcs, rcs = {}, {}, {}, {}, {}
    for beta in range(PB):
        bsl = slice(beta * BL, (beta + 1) * BL)
        xa = A[:, bsl, :, 0].transpose(2, 0, 1).reshape(N, CA)      # col=la*BL+b
        xu = train_init[bsl, :, :, 1].transpose(1, 2, 0).reshape(N, CU)  # col=t*BL+b
        xts[beta] = np.ascontiguousarray(
            np.concatenate([xa, xu], axis=1), dtype=np.float32)
    for g in range(PJ):
        wcs[g] = np.ascontiguousarray(Wneg[:, g * JL:(g + 1) * JL])
        pcs[g] = _coef_chunk(P, g)
        qcs[g] = _coef_chunk(Q, g)
        rcs[g] = _coef_chunk(R, g)

    for core in range(PB * PJ):
        beta, g = divmod(core, PJ)
        in_maps.append({
            "xt": xts[beta], "wc": wcs[g],
            "pc": pcs[g], "qc": qcs[g], "rc": rcs[g],
        })

    nc = _get_nc()
    res = run_bass_kernel_spmd(nc, in_maps, core_ids=list(range(PB * PJ)))
    kernel.last_results = res

    full = np.empty((B, NL, N), dtype=np.float32)
    for core in range(PB * PJ):
        beta, g = divmod(core, PJ)
        o = res.results[core]["out"]            # [300, 192], col = la*BL+b
        full[beta * BL:(beta + 1) * BL, :, g * JL:(g + 1) * JL] = (
            o.reshape(JL, NL, BL).transpose(2, 1, 0))
    return full.reshape(B, NL, N, 1)


# revision 9
# speedup vs baseline: 1.1484x; 1.1484x over previous
# Trainium2 Bass kernel for nn_FCM_series_1 (gnn_message_passing).
#
# Math (derived from the reference):
#   aggregate(X, WW)[l,b,j] = tanh(-sum_i X[l,b,i] * WW[i,j])
#   T_A  = aggregate(A, WW)                     (12 lags x B rows)
#   U[t] = aggregate(train_init[:,:,t,1], WW)   (13 unique rows per batch;
#          A_N_OLD[la] = U[la], A_0_NEW[la] = U[la+1])
#   out[b,la,j] = P[la,j]*T_A[la,b,j] + Q[la,j]*U[la+1,b,j] + R[la,j]*U[la,b,j]
# with host-computable coefficients
#   P[la,j] = 2 * lambd[la, j%200] / belta[la] * 3**fract[la]
#   Q[la,j] = 3 * lambd[la, j%200] * l[la, j%200] / belta[la]
#   R[la,j] = Q[la,j] * Gamma(a+1)/(6*Gamma(a-2))
#   belta[la] = sum_{k=0..3} Gamma(a+1)/(Gamma(k+1)*Gamma(a-k+1))
#
# Sharding over 8 cores: batch split x2 (16 each), output node dim j split x4
# (300 each). Per core one matmul chain: lhsT=W-chunk tiles, rhs=X^T tiles,
# PSUM-accumulated over 10 k-tiles of 120, in float32r (single-pass fp32 PE
# mode, 4x faster than fp32 LOW_HIGH). W is negated on the host so psum
# already holds -X@W; tanh on ScalarE; coefficient combine on VectorE with
# 0-stride broadcast APs; per-core [300,192] result re-assembled on the host.
#
# HBM layouts are host-repacked to partition-major so every DMA descriptor is
# one large contiguous run per partition; input DMAs are split between the two
# HWDGE queues (sync for W, scalar for X) to double aggregate DMA throughput.

import math

import numpy as np

LAG = 13
B = 32
N = 1200
H = 1.0 / 3.0

PB = 2          # batch shards
PJ = 4          # j shards
BL = B // PB    # 16 batches per core
JL = N // PJ    # 300 output nodes per core
NL = LAG - 1    # 12
CA = NL * BL    # 192 cols: T_A block, col = la*BL + b
CU = LAG * BL   # 208 cols: U block,  col = CA + t*BL + b
C = CA + CU     # 400 matmul moving cols
KT = 120        # contraction tile
NK = N // KT    # 10
JS = 100        # j subtile (psum partition dim)
NJ = JL // JS   # 3
NCH = 5         # input DMA chunks per tensor (2 k-tiles each)

_cached = None


def _gamma(x):
    return math.gamma(x)


def _build_nc():
    import concourse.bacc as bacc
    import concourse.mybir as mybir
    from concourse.tile import TileContext

    f32 = mybir.dt.float32
    f32r = mybir.dt.float32r
    nc = bacc.Bacc(None, target_bir_lowering=False)

    # partition-major repacked inputs (see kernel() for layouts)
    xt = nc.dram_tensor("xt", [KT, NK * C], f32r, kind="ExternalInput")
    wc = nc.dram_tensor("wc", [KT, NK * JL], f32r, kind="ExternalInput")
    coef = nc.dram_tensor("coef", [JS, 3 * NJ * NL], f32, kind="ExternalInput")
    out = nc.dram_tensor("out", [JL, CA], f32, kind="ExternalOutput")

    with TileContext(nc) as tc:
        with (
            tc.tile_pool(name="sb", bufs=1) as pool,
            tc.tile_pool(name="ps", bufs=1, space="PSUM") as pspool,
        ):
            w_all = pool.tile([KT, NK * JL], f32r, tag="w")
            x_all = pool.tile([KT, NK * C], f32r, tag="x")
            coef_all = pool.tile([JS, 3 * NJ * NL], f32, tag="coef")
            nc.sync.dma_start(out=coef_all[:], in_=coef[:, :])
            wch = (NK * JL) // NCH
            xch = (NK * C) // NCH
            for i in range(NCH):
                nc.sync.dma_start(
                    out=w_all[:, i * wch:(i + 1) * wch],
                    in_=wc[:, i * wch:(i + 1) * wch])
                nc.scalar.dma_start(
                    out=x_all[:, i * xch:(i + 1) * xch],
                    in_=xt[:, i * xch:(i + 1) * xch])

            ps = [pspool.tile([JS, C], f32, tag=f"ps{jt}", name=f"ps{jt}")
                  for jt in range(NJ)]
            for k in range(NK):
                for jt in range(NJ):
                    nc.tensor.matmul(
                        ps[jt][:],
                        w_all[:, k * JL + jt * JS:k * JL + (jt + 1) * JS],
                        x_all[:, k * C:(k + 1) * C],
                        start=(k == 0),
                        stop=(k == NK - 1),
                    )

            t_all = pool.tile([JS, NJ * C], f32, tag="t")
            for jt in range(NJ):
                # W was negated on the host, so psum = -(X @ W) already.
                nc.scalar.activation(
                    out=t_all[:, jt * C:(jt + 1) * C], in_=ps[jt][:],
                    func=mybir.ActivationFunctionType.Tanh,
                )

            res = pool.tile([JS, NJ * CA], f32, tag="res")
            tmp = pool.tile([JS, NJ * CA], f32, tag="tmp")
            t3 = t_all[:, :].rearrange("p (j c) -> p j c", j=NJ)
            tA = t3[:, :, 0:CA].rearrange("p j (l b) -> p j l b", b=BL)
            tU1 = t3[:, :, CA + BL:CA + CU].rearrange(
                "p j (l b) -> p j l b", b=BL)
            tU0 = t3[:, :, CA:CA + CA].rearrange("p j (l b) -> p j l b", b=BL)
            resv = res[:, :].rearrange("p (j l b) -> p j l b", j=NJ, b=BL)
            tmpv = tmp[:, :].rearrange("p (j l b) -> p j l b", j=NJ, b=BL)
            cofs = [
                coef_all[:, i * NJ * NL:(i + 1) * NJ * NL]
                .rearrange("p (j l) -> p j l", j=NJ)
                .broadcast_to([JS, NJ, NL, BL])
                for i in range(3)
            ]
            nc.vector.tensor_mul(resv, cofs[0], tA)
            nc.vector.tensor_mul(tmpv, cofs[1], tU1)
            nc.vector.tensor_add(res[:], res[:], tmp[:])
            nc.vector.tensor_mul(tmpv, cofs[2], tU0)
            nc.vector.tensor_add(res[:], res[:], tmp[:])

            nc.sync.dma_start(
                out=out.rearrange("(j p) c -> p j c", p=JS),
                in_=res[:, :].rearrange("p (j c) -> p j c", j=NJ))

    return nc


def _get_nc():
    global _cached
    if _cached is None:
        _cached = _build_nc()
        _cached.finalize()   # Bacc: runs reg alloc + codegen passes
    return _cached


def _host_coefs(alpha, fract, lambd, l):
    # All [12,...] fp32; compute in float64, cast at the end.
    a = alpha[:, 0].astype(np.float64)          # [12]
    f = fract[:, 0].astype(np.float64)          # [12]
    lam = lambd[:, 0, :, 0].astype(np.float64)  # [12, 200]
    ll = l[:, 0, :, 0].astype(np.float64)       # [12, 200]

    belta = np.zeros(NL)
    for la in range(NL):
        g_a1 = _gamma(a[la] + 1.0)
        belta[la] = sum(
            g_a1 / (_gamma(kk + 1.0) * _gamma(a[la] - kk + 1.0)) for kk in range(4)
        )
    cN = np.array([_gamma(a[la] + 1.0) / (6.0 * _gamma(a[la] - 2.0))
                   for la in range(NL)])

    # tile lambda/l from 200 -> 1200 (index n % 200)
    lam_t = np.tile(lam, (1, 6))                # [12, 1200]
    ll_t = np.tile(ll, (1, 6))                  # [12, 1200]

    inv_hf = (1.0 / H) ** f                     # 3**fract
    P = 2.0 * lam_t / belta[:, None] * inv_hf[:, None]
    Q = lam_t * ll_t / belta[:, None] / H
    R = Q * cN[:, None]
    return P.astype(np.float32), Q.astype(np.float32), R.astype(np.float32)


def kernel(A, WW, train_init, alpha, fract, lambd, l, A_y_list):
    from concourse.bass_utils import run_bass_kernel_spmd

    A = np.asarray(A, dtype=np.float32)
    WW = np.asarray(WW, dtype=np.float32)
    train_init = np.asarray(train_init, dtype=np.float32)

    P, Q, R = _host_coefs(
        np.asarray(alpha, np.float32), np.asarray(fract, np.float32),
        np.asarray(lambd, np.float32), np.asarray(l, np.float32))

    Wneg = -WW[:, :, 0]                         # [1200, 1200]

    xts, wcs, coefs = {}, {}, {}
    for beta in range(PB):
        bsl = slice(beta * BL, (beta + 1) * BL)
        xa = A[:, bsl, :, 0].transpose(2, 0, 1).reshape(N, CA)      # col=la*BL+b
        xu = train_init[bsl, :, :, 1].transpose(1, 2, 0).reshape(N, CU)  # col=t*BL+b
        XT = np.concatenate([xa, xu], axis=1)                       # [1200, 400]
        # partition-major: [KT, NK*C], col = k*C + c
        xts[beta] = np.ascontiguousarray(
            XT.reshape(NK, KT, C).transpose(1, 0, 2).reshape(KT, NK * C),
            dtype=np.float32)
    for g in range(PJ):
        gsl = slice(g * JL, (g + 1) * JL)
        # partition-major: [KT, NK*JL], col = k*JL + j
        wcs[g] = np.ascontiguousarray(
            Wneg[:, gsl].reshape(NK, KT, JL).transpose(1, 0, 2)
            .reshape(KT, NK * JL), dtype=np.float32)
        # coef [JS, 108]: col = kind*36 + jt*12 + la
        kinds = [M[:, gsl].reshape(NL, NJ, JS).transpose(2, 1, 0)
                 for M in (P, Q, R)]                                # [100, 3, 12]
        coefs[g] = np.ascontiguousarray(
            np.stack(kinds, axis=1).reshape(JS, 3 * NJ * NL), dtype=np.float32)

    in_maps = []
    for core in range(PB * PJ):
        beta, g = divmod(core, PJ)
        in_maps.append({"xt": xts[beta], "wc": wcs[g], "coef": coefs[g]})

    nc = _get_nc()
    res = run_bass_kernel_spmd(nc, in_maps, core_ids=list(range(PB * PJ)))
    kernel.last_results = res

    full = np.empty((B, NL, N), dtype=np.float32)
    for core in range(PB * PJ):
        beta, g = divmod(core, PJ)
        o = res.results[core]["out"]            # [300, 192], col = la*BL+b
        full[beta * BL:(beta + 1) * BL, :, g * JL:(g + 1) * JL] = (
            o.reshape(JL, NL, BL).transpose(2, 1, 0))
    return full.reshape(B, NL, N, 1)


# revision 10
# speedup vs baseline: 1.1785x; 1.0262x over previous
# Trainium2 Bass kernel for nn_FCM_series_1 (gnn_message_passing).
#
# Math (derived from the reference):
#   aggregate(X, WW)[l,b,j] = tanh(-sum_i X[l,b,i] * WW[i,j])
#   T_A  = aggregate(A, WW)                     (12 lags x B rows)
#   U[t] = aggregate(train_init[:,:,t,1], WW)   (13 unique rows per batch;
#          A_N_OLD[la] = U[la], A_0_NEW[la] = U[la+1])
#   out[b,la,j] = P[la,j]*T_A[la,b,j] + Q[la,j]*U[la+1,b,j] + R[la,j]*U[la,b,j]
# with host-computable coefficients
#   P[la,j] = 2 * lambd[la, j%200] / belta[la] * 3**fract[la]
#   Q[la,j] = 3 * lambd[la, j%200] * l[la, j%200] / belta[la]
#   R[la,j] = Q[la,j] * Gamma(a+1)/(6*Gamma(a-2))
#   belta[la] = sum_{k=0..3} Gamma(a+1)/(Gamma(k+1)*Gamma(a-k+1))
#
# Sharding over 8 cores: batch split x2 (16 each), output node dim j split x4
# (300 each). Per core one matmul chain: lhsT=W-chunk tiles, rhs=X^T tiles,
# PSUM-accumulated over 10 k-tiles of 120, in float32r (single-pass fp32 PE
# mode, 4x faster than fp32 LOW_HIGH). W is negated on the host so psum
# already holds -X@W; tanh on ScalarE; coefficient combine on VectorE with
# 0-stride broadcast APs; per-core [300,192] result re-assembled on the host.
#
# HBM layouts are host-repacked to partition-major so every DMA descriptor is
# one large contiguous run per partition; input DMAs are split between the two
# HWDGE queues (sync for W, scalar for X) to double aggregate DMA throughput.

import math

import numpy as np

LAG = 13
B = 32
N = 1200
H = 1.0 / 3.0

PB = 2          # batch shards
PJ = 4          # j shards
BL = B // PB    # 16 batches per core
JL = N // PJ    # 300 output nodes per core
NL = LAG - 1    # 12
CA = NL * BL    # 192 cols: T_A block, col = la*BL + b
CU = LAG * BL   # 208 cols: U block,  col = CA + t*BL + b
C = CA + CU     # 400 matmul moving cols
KT = 120        # contraction tile
NK = N // KT    # 10
JS = 100        # j subtile (psum partition dim)
NJ = JL // JS   # 3
NCH = 5         # input DMA chunks per tensor (2 k-tiles each)

_cached = None


def _gamma(x):
    return math.gamma(x)


def _build_nc():
    import concourse.bacc as bacc
    import concourse.mybir as mybir
    from concourse.tile import TileContext

    f32 = mybir.dt.float32
    f32r = mybir.dt.float32r
    nc = bacc.Bacc(None, target_bir_lowering=False)

    # partition-major repacked inputs (see kernel() for layouts)
    xt = nc.dram_tensor("xt", [KT, NK * C], f32r, kind="ExternalInput")
    wc = nc.dram_tensor("wc", [KT, NK * JL], f32r, kind="ExternalInput")
    coef = nc.dram_tensor("coef", [JS, 3 * NJ * NL], f32, kind="ExternalInput")
    out = nc.dram_tensor("out", [JL, CA], f32, kind="ExternalOutput")

    with TileContext(nc) as tc:
        with (
            tc.tile_pool(name="sb", bufs=1) as pool,
            tc.tile_pool(name="ps", bufs=1, space="PSUM") as pspool,
        ):
            KPC = NK // NCH          # k-tiles per DMA chunk (2)
            wch = KPC * JL
            xch = KPC * C
            wt, xtt = [], []
            for i in range(NCH):
                w_tile = pool.tile([KT, wch], f32r, tag="w", bufs=NCH,
                                   name=f"w{i}")
                nc.sync.dma_start(
                    out=w_tile[:], in_=wc[:, i * wch:(i + 1) * wch])
                wt.append(w_tile)
                x_tile = pool.tile([KT, xch], f32r, tag="x", bufs=NCH,
                                   name=f"x{i}")
                nc.scalar.dma_start(
                    out=x_tile[:], in_=xt[:, i * xch:(i + 1) * xch])
                xtt.append(x_tile)
            coef_all = pool.tile([JS, 3 * NJ * NL], f32, tag="coef")
            nc.sync.dma_start(out=coef_all[:], in_=coef[:, :])

            ps = [pspool.tile([JS, C], f32, tag=f"ps{jt}", name=f"ps{jt}")
                  for jt in range(NJ)]
            for k in range(NK):
                ci, kk = divmod(k, KPC)
                for jt in range(NJ):
                    nc.tensor.matmul(
                        ps[jt][:],
                        wt[ci][:, kk * JL + jt * JS:kk * JL + (jt + 1) * JS],
                        xtt[ci][:, kk * C:(kk + 1) * C],
                        start=(k == 0),
                        stop=(k == NK - 1),
                    )

            t_all = pool.tile([JS, NJ * C], f32, tag="t")
            for jt in range(NJ):
                # W was negated on the host, so psum = -(X @ W) already.
                nc.scalar.activation(
                    out=t_all[:, jt * C:(jt + 1) * C], in_=ps[jt][:],
                    func=mybir.ActivationFunctionType.Tanh,
                )

            res = pool.tile([JS, NJ * CA], f32, tag="res")
            tmp = pool.tile([JS, NJ * CA], f32, tag="tmp")
            t3 = t_all[:, :].rearrange("p (j c) -> p j c", j=NJ)
            tA = t3[:, :, 0:CA].rearrange("p j (l b) -> p j l b", b=BL)
            tU1 = t3[:, :, CA + BL:CA + CU].rearrange(
                "p j (l b) -> p j l b", b=BL)
            tU0 = t3[:, :, CA:CA + CA].rearrange("p j (l b) -> p j l b", b=BL)
            resv = res[:, :].rearrange("p (j l b) -> p j l b", j=NJ, b=BL)
            tmpv = tmp[:, :].rearrange("p (j l b) -> p j l b", j=NJ, b=BL)
            cofs = [
                coef_all[:, i * NJ * NL:(i + 1) * NJ * NL]
                .rearrange("p (j l) -> p j l", j=NJ)
                .broadcast_to([JS, NJ, NL, BL])
                for i in range(3)
            ]
            nc.vector.tensor_mul(resv, cofs[0], tA)
            nc.vector.tensor_mul(tmpv, cofs[1], tU1)
            nc.vector.tensor_add(res[:], res[:], tmp[:])
            nc.vector.tensor_mul(tmpv, cofs[2], tU0)
            nc.vector.tensor_add(res[:], res[:], tmp[:])

            nc.sync.dma_start(
                out=out.rearrange("(j p) c -> p j c", p=JS),
                in_=res[:, :].rearrange("p (j c) -> p j c", j=NJ))

    return nc


def _get_nc():
    global _cached
    if _cached is None:
        _cached = _build_nc()
        _cached.finalize()   # Bacc: runs reg alloc + codegen passes
    return _cached


def _host_coefs(alpha, fract, lambd, l):
    # All [12,...] fp32; compute in float64, cast at the end.
    a = alpha[:, 0].astype(np.float64)          # [12]
    f = fract[:, 0].astype(np.float64)          # [12]
    lam = lambd[:, 0, :, 0].astype(np.float64)  # [12, 200]
    ll = l[:, 0, :, 0].astype(np.float64)       # [12, 200]

    belta = np.zeros(NL)
    for la in range(NL):
        g_a1 = _gamma(a[la] + 1.0)
        belta[la] = sum(
            g_a1 / (_gamma(kk + 1.0) * _gamma(a[la] - kk + 1.0)) for kk in range(4)
        )
    cN = np.array([_gamma(a[la] + 1.0) / (6.0 * _gamma(a[la] - 2.0))
                   for la in range(NL)])

    # tile lambda/l from 200 -> 1200 (index n % 200)
    lam_t = np.tile(lam, (1, 6))                # [12, 1200]
    ll_t = np.tile(ll, (1, 6))                  # [12, 1200]

    inv_hf = (1.0 / H) ** f                     # 3**fract
    P = 2.0 * lam_t / belta[:, None] * inv_hf[:, None]
    Q = lam_t * ll_t / belta[:, None] / H
    R = Q * cN[:, None]
    return P.astype(np.float32), Q.astype(np.float32), R.astype(np.float32)


def kernel(A, WW, train_init, alpha, fract, lambd, l, A_y_list):
    from concourse.bass_utils import run_bass_kernel_spmd

    A = np.asarray(A, dtype=np.float32)
    WW = np.asarray(WW, dtype=np.float32)
    train_init = np.asarray(train_init, dtype=np.float32)

    P, Q, R = _host_coefs(
        np.asarray(alpha, np.float32), np.asarray(fract, np.float32),
        np.asarray(lambd, np.float32), np.asarray(l, np.float32))

    Wneg = -WW[:, :, 0]                         # [1200, 1200]

    xts, wcs, coefs = {}, {}, {}
    for beta in range(PB):
        bsl = slice(beta * BL, (beta + 1) * BL)
        xa = A[:, bsl, :, 0].transpose(2, 0, 1).reshape(N, CA)      # col=la*BL+b
        xu = train_init[bsl, :, :, 1].transpose(1, 2, 0).reshape(N, CU)  # col=t*BL+b
        XT = np.concatenate([xa, xu], axis=1)                       # [1200, 400]
        # partition-major: [KT, NK*C], col = k*C + c
        xts[beta] = np.ascontiguousarray(
            XT.reshape(NK, KT, C).transpose(1, 0, 2).reshape(KT, NK * C),
            dtype=np.float32)
    for g in range(PJ):
        gsl = slice(g * JL, (g + 1) * JL)
        # partition-major: [KT, NK*JL], col = k*JL + j
        wcs[g] = np.ascontiguousarray(
            Wneg[:, gsl].reshape(NK, KT, JL).transpose(1, 0, 2)
            .reshape(KT, NK * JL), dtype=np.float32)
        # coef [JS, 108]: col = kind*36 + jt*12 + la
        kinds = [M[:, gsl].reshape(NL, NJ, JS).transpose(2, 1, 0)
                 for M in (P, Q, R)]                                # [100, 3, 12]
        coefs[g] = np.ascontiguousarray(
            np.stack(kinds, axis=1).reshape(JS, 3 * NJ * NL), dtype=np.float32)

    in_maps = []
    for core in range(PB * PJ):
        beta, g = divmod(core, PJ)
        in_maps.append({"xt": xts[beta], "wc": wcs[g], "coef": coefs[g]})

    nc = _get_nc()
    res = run_bass_kernel_spmd(nc, in_maps, core_ids=list(range(PB * PJ)))
    kernel.last_results = res

    full = np.empty((B, NL, N), dtype=np.float32)
    for core in range(PB * PJ):
        beta, g = divmod(core, PJ)
        o = res.results[core]["out"]            # [300, 192], col = la*BL+b
        full[beta * BL:(beta + 1) * BL, :, g * JL:(g + 1) * JL] = (
            o.reshape(JL, NL, BL).transpose(2, 1, 0))
    return full.reshape(B, NL, N, 1)
